# revision 3
# baseline (speedup 1.0000x reference)
"""LOGG3D_ATTN kernel v3 for the fake_nrt/axon TRN2 backend.

The graded metric (differential wall over in-program repeat) is dominated by
per-instruction NEFF load/translate cost on this backend, with DMA costing
~90us per descriptor (one per partition row). Design rules:
  - minimize instruction count (matmul moving-N at the 512-f32 ISA cap,
    activation/stt chunks of FD=4096 = the whole PSUM)
  - fuse reductions into compute instructions via accum_out
  - keep per-iteration DMA tiny: combine partials, transpose [128,24] ->
    [12,256] on TensorE, single 12-descriptor DMA out.

Math (per core, rows sharded, R=1536 rows = 12 blocks of 128):
  s[i, j] = feats_i . feats_j                  TensorE  (24 mm of N=512/block)
  e = exp(s/4)   + accum -> denom_i = sum_j e  ScalarE  (3 acts of FD=4096/block)
  m = e * s      + accum -> numer_i = sum_j m  VectorE  scalar_tensor_tensor
  weights_i = sigmoid(numer_i / denom_i)  [host]  == sigmoid(ctx_i . f_i) since
  ctx_i . f_i = sum_j softmax(s/4)_ij s_ij.  exp needs no row-max shift:
  |s|/4 <= ~11.3 here, safely inside fp32 range.
Host epilogue: sigmoid, top-k (a permutation when topK == 1), SOP outer
product pooling, L2 normalize -- O(N D^2), matches the reference.
"""

import time

import numpy as np

import concourse.bacc as bacc
import concourse.mybir as mybir
import concourse.tile as tile
from concourse import bass_utils

N_POINTS = 12288
FEAT_DIM = 16
N_CORES = 8

last_profile = {}
_program_cache = {}

f32 = mybir.dt.float32
f32r = mybir.dt.float32r
EXP = mybir.ActivationFunctionType.Exp
MULT = mybir.AluOpType.mult
ADD = mybir.AluOpType.add
AXX = mybir.AxisListType.X


def build_program(N, R, D=FEAT_DIM, FD=4096, e_bufs=3, m_bufs=2, repeat=1,
                  phases="full"):
    """Per-core SPMD program -> [12, 256] f32: row 0..11 = block, cols 0:128
    denom, cols 128:256 numer (per partition row within the block)."""
    key = (N, R, D, FD, e_bufs, m_bufs, repeat, phases)
    if key in _program_cache:
        return _program_cache[key]

    assert R % 128 == 0 and N % FD == 0
    B = R // 128
    NCH = N // FD
    mmN = 512
    QN = FD // mmN

    nc = bacc.Bacc("TRN2", target_bir_lowering=False, debug=False)

    featsT_d = nc.dram_tensor("featsT", [D, N], f32r, kind="ExternalInput")
    myT_d = nc.dram_tensor("myT", [D, R], f32r, kind="ExternalInput")
    ident_d = nc.dram_tensor("ident", [128, 128], f32, kind="ExternalInput")
    out_d = nc.dram_tensor("nd_out", [B, 256], f32, kind="ExternalOutput")

    with tile.TileContext(nc) as tc:
        with (
            tc.tile_pool(name="const", bufs=1) as cpool,
            tc.tile_pool(name="st", bufs=1, space="PSUM") as st_pool,
            tc.tile_pool(name="e", bufs=e_bufs) as e_pool,
            tc.tile_pool(name="m", bufs=m_bufs) as m_pool,
            tc.tile_pool(name="acc", bufs=2) as a_pool,
            tc.tile_pool(name="o", bufs=2) as o_pool,
        ):
            featsT_sb = cpool.tile([D, N], f32r)
            nc.sync.dma_start(featsT_sb[:], featsT_d[:])
            myT_sb = cpool.tile([D, R], f32r)
            nc.sync.dma_start(myT_sb[:], myT_d[:])
            ident_sb = cpool.tile([128, 128], f32)
            nc.sync.dma_start(ident_sb[:], ident_d[:])

            for rep in range(repeat):
                dacc = a_pool.tile([128, B, NCH], f32, tag="dacc", name="dacc")
                nacc = a_pool.tile([128, B, NCH], f32, tag="nacc", name="nacc")
                for b in range(B):
                    lhs = myT_sb[:, b * 128:(b + 1) * 128]
                    for kc in range(NCH):
                        st = st_pool.tile([128, FD], f32, tag="st", name="st")
                        for q in range(QN):
                            c0 = kc * FD + q * mmN
                            nc.tensor.matmul(
                                st[:, q * mmN:(q + 1) * mmN],
                                lhs,
                                featsT_sb[:, c0:c0 + mmN],
                                start=True,
                                stop=True,
                            )
                        e_t = e_pool.tile([128, FD], f32, tag="e", name="e_t")
                        nc.scalar.activation(
                            e_t[:], st[:], EXP, scale=0.25,
                            accum_out=dacc[:, b, kc:kc + 1],
                        )
                        m_t = m_pool.tile([128, FD], f32, tag="m", name="m_t")
                        nc.vector.scalar_tensor_tensor(
                            m_t[:], e_t[:], 1.0, st[:], MULT, MULT,
                            accum_out=nacc[:, b, kc:kc + 1],
                        )
                # combine chunk partials: [128, B, NCH] -> [128, B]
                ndc = o_pool.tile([128, 2 * B], f32, tag="ndc", name="ndc")
                nc.vector.tensor_reduce(ndc[:, 0:B], dacc[:], AXX, ADD)
                nc.vector.tensor_reduce(ndc[:, B:2 * B], nacc[:], AXX, ADD)
                # transpose [128, B]x2 -> [B, 256] so the out DMA is B descriptors
                tps = st_pool.tile([B, 256], f32, tag="st", name="tps")
                nc.tensor.transpose(tps[:, 0:128], ndc[:, 0:B], ident_sb[:])
                nc.tensor.transpose(tps[:, 128:256], ndc[:, B:2 * B], ident_sb[:])
                osm = o_pool.tile([B, 256], f32, tag="osm", name="osm")
                nc.vector.tensor_copy(osm[:], tps[:])
                nc.sync.dma_start(out_d[:], osm[:])

    nc.compile()
    _program_cache[key] = nc
    return nc


def _make_in_maps(feats, N, R, D):
    featsT = np.ascontiguousarray(feats.T).astype(np.float32)          # [D, N]
    ident = np.eye(128, dtype=np.float32)
    in_maps = []
    for c in range(N_CORES):
        myT = np.ascontiguousarray(featsT[:, c * R:(c + 1) * R])
        in_maps.append({"featsT": featsT, "myT": myT, "ident": ident})
    return in_maps


def _numer_denom_on_device(feats, N, R, D, FD=4096):
    """Returns (numer, denom) float64 [N]."""
    nc = build_program(N, R, D=D, FD=FD)
    in_maps = _make_in_maps(feats, N, R, D)

    res = None
    for attempt in range(3):
        try:
            res = bass_utils.run_bass_kernel_spmd(nc, in_maps, list(range(N_CORES)))
            break
        except Exception:
            if attempt == 2:
                raise
            time.sleep(5.0 * (attempt + 1))

    global last_profile
    last_profile = {"exec_time_ns": res.exec_time_ns}

    B = R // 128
    denom = np.empty(N, np.float64)
    numer = np.empty(N, np.float64)
    for c in range(N_CORES):
        nd = np.asarray(res.results[c]["nd_out"], dtype=np.float64)  # [B, 256]
        denom[c * R:(c + 1) * R] = nd[:, 0:128].reshape(R)
        numer[c * R:(c + 1) * R] = nd[:, 128:256].reshape(R)
    return numer, denom


def _kernel_impl(feats, topK, N, D, FD=4096):
    feats = np.asarray(feats, dtype=np.float32)
    R = N // N_CORES
    numer, denom = _numer_denom_on_device(feats, N, R, D, FD=FD)

    w = 1.0 / (1.0 + np.exp(-(numer / denom)))                          # [N]

    weighted = feats * w[:, None].astype(np.float32)                    # [N, D]
    k = int(N * np.asarray(topK).item())
    if k >= N:
        sel = weighted
    else:
        top_idx = np.argsort(-w, kind="stable")[:k]
        sel = weighted[top_idx]
    so = (sel.T.astype(np.float32) @ sel.astype(np.float32)) / np.float32(max(k, 1))
    out = so.reshape(1, -1).astype(np.float32)
    nrm = np.linalg.norm(out, axis=-1, keepdims=True).astype(np.float32)
    out = out / nrm
    return out.astype(np.float32)


def kernel(feats, topK):
    return _kernel_impl(feats, topK, N_POINTS, FEAT_DIM)


# revision 9
# speedup vs baseline: 1.0076x; 1.0076x over previous
"""LOGG3D_ATTN kernel v3 for the fake_nrt/axon TRN2 backend.

The graded metric (differential wall over in-program repeat) is dominated by
per-instruction NEFF load/translate cost on this backend, with DMA costing
~90us per descriptor (one per partition row). Design rules:
  - minimize instruction count (matmul moving-N at the 512-f32 ISA cap,
    activation/stt chunks of FD=4096 = the whole PSUM)
  - fuse reductions into compute instructions via accum_out
  - keep per-iteration DMA tiny: combine partials, transpose [128,24] ->
    [12,256] on TensorE, single 12-descriptor DMA out.

Math (per core, rows sharded, R=1536 rows = 12 blocks of 128):
  s[i, j] = feats_i . feats_j                   TensorE (24 mm of N=512/block)
  Z+- = sum_j exp((0.25 +- h) s_ij)  via accum  ScalarE (2 acts of FD=4096/chunk)
  denom_i = (Z+ + Z-)/2,  numer_i = (Z+ - Z-)/(2h)   [central difference of
  Z(a) = sum_j exp(a s_ij); truncation O(h^2) only matters on rows whose
  sigmoid is saturated anyway]
  weights_i = sigmoid(numer_i / denom_i)  [host]  == sigmoid(ctx_i . f_i) since
  ctx_i . f_i = sum_j softmax(s/4)_ij s_ij.  exp needs no row-max shift:
  (0.25+h)|s| <= ~12 here, safely inside fp32 range.  Using two ScalarE
  activations instead of act+VectorE-multiply keeps the whole per-chunk flow
  on one consumer engine: Tile emits 39 semaphore instructions/iter vs 111.
Host epilogue: sigmoid, top-k (a permutation when topK == 1), SOP outer
product pooling, L2 normalize -- O(N D^2), matches the reference.
"""

import time

import numpy as np

import concourse.bacc as bacc
import concourse.mybir as mybir
import concourse.tile as tile
from concourse import bass_utils

N_POINTS = 12288
FEAT_DIM = 16
N_CORES = 8
DEFAULT_MODE = "fd2"

last_profile = {}
_program_cache = {}

f32 = mybir.dt.float32
f32r = mybir.dt.float32r
EXP = mybir.ActivationFunctionType.Exp
MULT = mybir.AluOpType.mult
ADD = mybir.AluOpType.add
AXX = mybir.AxisListType.X


def build_program(N, R, D=FEAT_DIM, FD=4096, e_bufs=3, m_bufs=2, repeat=1,
                  phases="full", mode="stt", fdh=1.0 / 64.0):
    """Per-core SPMD program -> [12, 256] f32: row 0..11 = block, cols 0:128
    and 128:256 are the two per-row accumulators (per partition row within
    the block): mode "stt" -> (denom, numer); mode "fd2" -> (Z+, Z-) at
    activation scales 0.25 +/- fdh (host forms denom=(Z++Z-)/2 and
    numer=(Z+-Z-)/(2 fdh) -- central difference of Z(a)=sum exp(a*s))."""
    key = (N, R, D, FD, e_bufs, m_bufs, repeat, phases, mode, fdh)
    if key in _program_cache:
        return _program_cache[key]

    assert R % 128 == 0 and N % FD == 0
    B = R // 128
    NCH = N // FD
    mmN = 512
    QN = FD // mmN

    nc = bacc.Bacc("TRN2", target_bir_lowering=False, debug=False)

    featsT_d = nc.dram_tensor("featsT", [D, N], f32r, kind="ExternalInput")
    myT_d = nc.dram_tensor("myT", [D, R], f32r, kind="ExternalInput")
    ident_d = nc.dram_tensor("ident", [128, 128], f32, kind="ExternalInput")
    out_d = nc.dram_tensor("nd_out", [B, 256], f32, kind="ExternalOutput")

    with tile.TileContext(nc) as tc:
        with (
            tc.tile_pool(name="const", bufs=1) as cpool,
            tc.tile_pool(name="st", bufs=1, space="PSUM") as st_pool,
            tc.tile_pool(name="e", bufs=e_bufs) as e_pool,
            tc.tile_pool(name="m", bufs=m_bufs) as m_pool,
            tc.tile_pool(name="acc", bufs=2) as a_pool,
            tc.tile_pool(name="o", bufs=2) as o_pool,
        ):
            featsT_sb = cpool.tile([D, N], f32r)
            nc.sync.dma_start(featsT_sb[:], featsT_d[:])
            myT_sb = cpool.tile([D, R], f32r)
            nc.sync.dma_start(myT_sb[:], myT_d[:])
            ident_sb = cpool.tile([128, 128], f32)
            nc.sync.dma_start(ident_sb[:], ident_d[:])

            for rep in range(repeat):
                dacc = a_pool.tile([128, B, NCH], f32, tag="dacc", name="dacc")
                nacc = a_pool.tile([128, B, NCH], f32, tag="nacc", name="nacc")
                for b in range(B):
                    lhs = myT_sb[:, b * 128:(b + 1) * 128]
                    for kc in range(NCH):
                        st = st_pool.tile([128, FD], f32, tag="st", name="st")
                        for q in range(QN):
                            c0 = kc * FD + q * mmN
                            nc.tensor.matmul(
                                st[:, q * mmN:(q + 1) * mmN],
                                lhs,
                                featsT_sb[:, c0:c0 + mmN],
                                start=True,
                                stop=True,
                            )
                        if mode == "stt":
                            e_t = e_pool.tile([128, FD], f32, tag="e", name="e_t")
                            nc.scalar.activation(
                                e_t[:], st[:], EXP, scale=0.25,
                                accum_out=dacc[:, b, kc:kc + 1],
                            )
                            m_t = m_pool.tile([128, FD], f32, tag="m", name="m_t")
                            nc.vector.scalar_tensor_tensor(
                                m_t[:], e_t[:], 1.0, st[:], MULT, MULT,
                                accum_out=nacc[:, b, kc:kc + 1],
                            )
                        else:  # fd2: both accumulators from ScalarE only
                            e_t = e_pool.tile([128, FD], f32, tag="e", name="e_t")
                            nc.scalar.activation(
                                e_t[:], st[:], EXP, scale=0.25 + fdh,
                                accum_out=dacc[:, b, kc:kc + 1],
                            )
                            m_t = m_pool.tile([128, FD], f32, tag="m", name="m_t")
                            nc.scalar.activation(
                                m_t[:], st[:], EXP, scale=0.25 - fdh,
                                accum_out=nacc[:, b, kc:kc + 1],
                            )
                # combine chunk partials: [128, B, NCH] -> [128, B]
                ndc = o_pool.tile([128, 2 * B], f32, tag="ndc", name="ndc")
                nc.vector.tensor_reduce(ndc[:, 0:B], dacc[:], AXX, ADD)
                nc.vector.tensor_reduce(ndc[:, B:2 * B], nacc[:], AXX, ADD)
                # transpose [128, B]x2 -> [B, 256] so the out DMA is B descriptors
                tps = st_pool.tile([B, 256], f32, tag="st", name="tps")
                nc.tensor.transpose(tps[:, 0:128], ndc[:, 0:B], ident_sb[:])
                nc.tensor.transpose(tps[:, 128:256], ndc[:, B:2 * B], ident_sb[:])
                osm = o_pool.tile([B, 256], f32, tag="osm", name="osm")
                nc.vector.tensor_copy(osm[:], tps[:])
                nc.sync.dma_start(out_d[:], osm[:])

    nc.compile()
    _program_cache[key] = nc
    return nc


def _make_in_maps(feats, N, R, D):
    featsT = np.ascontiguousarray(feats.T).astype(np.float32)          # [D, N]
    ident = np.eye(128, dtype=np.float32)
    in_maps = []
    for c in range(N_CORES):
        myT = np.ascontiguousarray(featsT[:, c * R:(c + 1) * R])
        in_maps.append({"featsT": featsT, "myT": myT, "ident": ident})
    return in_maps


def _numer_denom_on_device(feats, N, R, D, FD=4096, mode="stt", fdh=1.0 / 64.0):
    """Returns (numer, denom) float64 [N]."""
    nc = build_program(N, R, D=D, FD=FD, mode=mode, fdh=fdh)
    in_maps = _make_in_maps(feats, N, R, D)

    res = None
    for attempt in range(3):
        try:
            res = bass_utils.run_bass_kernel_spmd(nc, in_maps, list(range(N_CORES)))
            break
        except Exception:
            if attempt == 2:
                raise
            time.sleep(5.0 * (attempt + 1))

    global last_profile
    last_profile = {"exec_time_ns": res.exec_time_ns}

    B = R // 128
    denom = np.empty(N, np.float64)
    numer = np.empty(N, np.float64)
    for c in range(N_CORES):
        nd = np.asarray(res.results[c]["nd_out"], dtype=np.float64)  # [B, 256]
        a = nd[:, 0:128].reshape(R)
        bb = nd[:, 128:256].reshape(R)
        if mode == "stt":
            denom[c * R:(c + 1) * R] = a
            numer[c * R:(c + 1) * R] = bb
        else:  # fd2: a = Z(0.25+h), bb = Z(0.25-h)
            denom[c * R:(c + 1) * R] = (a + bb) / 2.0
            numer[c * R:(c + 1) * R] = (a - bb) / (2.0 * fdh)
    return numer, denom


def _kernel_impl(feats, topK, N, D, FD=4096, mode=None):
    if mode is None:
        mode = DEFAULT_MODE
    feats = np.asarray(feats, dtype=np.float32)
    R = N // N_CORES
    numer, denom = _numer_denom_on_device(feats, N, R, D, FD=FD, mode=mode)

    w = 1.0 / (1.0 + np.exp(-(numer / denom)))                          # [N]

    weighted = feats * w[:, None].astype(np.float32)                    # [N, D]
    k = int(N * np.asarray(topK).item())
    if k >= N:
        sel = weighted
    else:
        top_idx = np.argsort(-w, kind="stable")[:k]
        sel = weighted[top_idx]
    so = (sel.T.astype(np.float32) @ sel.astype(np.float32)) / np.float32(max(k, 1))
    out = so.reshape(1, -1).astype(np.float32)
    nrm = np.linalg.norm(out, axis=-1, keepdims=True).astype(np.float32)
    out = out / nrm
    return out.astype(np.float32)


def kernel(feats, topK):
    return _kernel_impl(feats, topK, N_POINTS, FEAT_DIM)
